# revision 36
# baseline (speedup 1.0000x reference)
import sys
sys.path.insert(0, '/opt/trn_rl_repo')
import numpy as np
import ml_dtypes

import concourse.bass as bass
import concourse.tile as tile
from concourse import bacc, mybir
from concourse.bass_utils import run_bass_kernel_spmd

# ---------------- problem constants (hardcoded per spec) ----------------
NTOT = 1_000_000          # total elements (X is [2, NTOT])
NCORES = 8
Q = 8                     # quadrature nodes (optimized for tanh/ADF, see _quad_consts)
G = 128 // Q              # element groups packed per partition column (16)
F = 512                   # free-dim elements per group per matmul (1 PSUM bank fp32)
EPT = G * F               # elements per tile (8192)
NC_ELEM = 131072          # per-core padded element count
T = NC_ELEM // EPT        # tiles per core (16)
STAGE = 128 // G          # tiles per output stage (8)
NSTAGES = T // STAGE      # 2
CH = NC_ELEM // (128 * F) # phase-1 chunks (2)
NPAD = NC_ELEM * NCORES
LAG = 2                   # software pipeline depth in tile-pairs

F32 = mybir.dt.float32
BF16 = mybir.dt.bfloat16

# 8-node symmetric quadrature for E[tanh(mu + sqrt2 s x)] / E[tanh^2], jointly
# optimized offline over mu in [0,1], s^2 in [0,1] with nodes AND weights
# constrained to the bf16 grid (greedy sequential quantization); separate
# weight sets for the first and second moment. ~3.1e-4 frob error vs the
# 128-node Gauss-Hermite reference (Gauss-Hermite-8 itself gives 7.9e-3).
_XH = [1.96875, 1.25, 0.703125, 0.2275390625]          # descending half-nodes
_W1H = [0.0096435546875, 0.0732421875, 0.1708984375, 0.24609375]
_W2H = [0.01055908203125, 0.0712890625, 0.1728515625, 0.2451171875]


def _quad_consts():
    x = np.array([-v for v in _XH] + _XH[::-1], dtype=np.float64)   # ascending
    w1 = np.array(_W1H + _W1H[::-1], dtype=np.float64)
    w2 = np.array(_W2H + _W2H[::-1], dtype=np.float64)

    # expansion lhsT: [2G, 128]; rhs partition p = g*2 + j (j: 0=mu, 1=std')
    # z partition m = g*Q + q
    E = np.zeros((2 * G, 128), dtype=np.float32)
    for m in range(128):
        g, q = divmod(m, Q)
        E[g * 2 + 0, m] = 1.0
        E[g * 2 + 1, m] = x[q]
    # reduction lhsT (M=64, weighted): slot j = s % 4 selects which 16-col band
    # carries the weights; out partition base is 64*(s//4).
    # RED packs [R1_0..R1_3 | R2_0..R2_3], each [128, 64].
    R = np.zeros((128, 8 * 64), dtype=np.float32)
    for j in range(4):
        for k in range(128):
            g, q = divmod(k, Q)
            R[k, j * 64 + j * G + g] = w1[q]
            R[k, (4 + j) * 64 + j * G + g] = w2[q]
    E4 = np.vstack([E, E, E, E])  # rhs slices at base partitions 0/32/64/96 reuse it
    return E4.astype(ml_dtypes.bfloat16), R.astype(ml_dtypes.bfloat16)


def _dram_ap(t_ap: bass.AP, offset: int, pattern) -> bass.AP:
    return bass.AP(tensor=t_ap.tensor, offset=offset, ap=[list(p) for p in pattern])


def build_graph():
    nc = bacc.Bacc("TRN2", target_bir_lowering=False, debug=False, num_devices=NCORES)
    X = nc.dram_tensor("X", [2, NC_ELEM], F32, kind="ExternalInput").ap()
    EXP = nc.dram_tensor("EXP", [8 * G, 128], BF16, kind="ExternalInput").ap()
    RED = nc.dram_tensor("RED", [128, 8 * 64], BF16, kind="ExternalInput").ap()
    OUT = nc.dram_tensor("out", [2, NC_ELEM], F32, kind="ExternalOutput").ap()

    with tile.TileContext(nc) as tc:
        with tc.tile_pool(name="consts", bufs=1) as consts, \
             tc.tile_pool(name="phase1", bufs=1) as ph1, \
             tc.tile_pool(name="rhs", bufs=3) as rpool, \
             tc.tile_pool(name="acts", bufs=2) as apool, \
             tc.tile_pool(name="stage", bufs=2) as spool, \
             tc.tile_pool(name="zps", bufs=2, space="PSUM") as zpool, \
             tc.tile_pool(name="mps", bufs=2, space="PSUM") as mpool:

            mu_f = ph1.tile([128, CH, F], F32)
            var_f = ph1.tile([128, CH, F], F32)
            for c in range(CH):
                nc.sync.dma_start(mu_f[:, c, :],
                                  _dram_ap(X, c * 128 * F, [[F, 128], [1, F]]))
                nc.gpsimd.dma_start(var_f[:, c, :],
                                    _dram_ap(X, NC_ELEM + c * 128 * F, [[F, 128], [1, F]]))

            e_sb = consts.tile([8 * G, 128], BF16)
            nc.scalar.dma_start(e_sb[:], EXP)
            r_sb = consts.tile([128, 8 * 64], BF16)
            nc.scalar.dma_start(r_sb[:], RED)
            wtiny = consts.tile([128, F], BF16)
            nc.vector.memset(wtiny[:], 0.001)

            # ---- warmup: open the PE clock gate while inputs stream in;
            # the last few depend on phase-1 data so PE activity continues
            # seamlessly into the first real z-matmul (no re-throttle gap).
            wm = zpool.tile([128, 2, F], F32, tag="z")
            for _ in range(12):
                nc.tensor.matmul(wm[:, 0, :], wtiny[:, 0:128], wtiny[:],
                                 start=True, stop=True, skip_group_check=True)

            # ---- phase 1: load X; msd[:, 0]=mu (bf16), msd[:, 1]=sqrt(2*var) (bf16)
            msd = ph1.tile([128, 2, CH, F], BF16)
            for c in range(CH):
                nc.vector.tensor_copy(msd[:, 0, c, :], mu_f[:, c, :])
                nc.scalar.activation(msd[:, 1, c, :], var_f[:, c, :],
                                     mybir.ActivationFunctionType.Sqrt, scale=2.0)

            for _ in range(4):
                nc.tensor.matmul(wm[:, 1, :], wtiny[0:2 * G, 0:128],
                                 msd[0:2 * G, 0, 0, 0:F].bitcast(BF16),
                                 start=True, stop=True, skip_group_check=True)

            # ---- main loop: software-pipelined; tile-pairs share one 2-bank
            # PSUM z tile so ACT/DVE process [128, 2F] spans.
            NP = T // 2
            z_tiles = [None] * NP
            stage_tiles = {}

            def emit_front(p):
                # tiles 2p, 2p+1 sit at contiguous partition ranges of msd
                s0 = (2 * p) % STAGE
                c = (2 * p) // STAGE
                rhs_p = rpool.tile([4 * G, F], BF16, tag="rhs")
                nc.gpsimd.dma_start(rhs_p[:], msd[s0 * G:(s0 + 2) * G, :, c, :])
                z_p = zpool.tile([128, 2, F], F32, tag="z")
                for h in range(2):
                    b = h * 2 * G
                    nc.tensor.matmul(z_p[:, h, :],
                                     e_sb[b:b + 2 * G, :],
                                     rhs_p[b:b + 2 * G, :],
                                     start=True, stop=True, skip_group_check=True,
                                     tile_position=(b, 0))
                z_tiles[p] = z_p

            def emit_epilogue(st):
                m1_stage, m2_stage = stage_tiles[st]
                m1_sb = spool.tile([128, F], F32, tag="m1sb")
                nc.vector.tensor_copy(m1_sb[:], m1_stage[:])
                sq = spool.tile([128, F], F32, tag="sq")
                nc.vector.tensor_mul(sq[:], m1_sb[:], m1_sb[:])
                var_t = spool.tile([128, F], F32, tag="var")
                nc.vector.tensor_sub(var_t[:], m2_stage[:], sq[:])
                off = st * 128 * F
                nc.sync.dma_start(_dram_ap(OUT, off, [[F, 128], [1, F]]), m1_sb[:])
                nc.scalar.dma_start(_dram_ap(OUT, NC_ELEM + off, [[F, 128], [1, F]]), var_t[:])

            def emit_back(p):
                z_p = z_tiles[p]
                a_p = apool.tile([128, 2, F], BF16, tag="a")
                nc.scalar.activation(a_p[:], z_p[:], mybir.ActivationFunctionType.Tanh)
                a2_p = apool.tile([128, 2, F], BF16, tag="a2")
                nc.vector.tensor_mul(a2_p[:], a_p[:], a_p[:])
                for h in range(2):
                    t = 2 * p + h
                    st, s = divmod(t, STAGE)
                    if s == 0:
                        m1s_new = mpool.tile([128, F], F32, tag="m1s")
                        m2s_new = mpool.tile([128, F], F32, tag="m2s")
                        stage_tiles[st] = (m1s_new, m2s_new)
                    m1_stage, m2_stage = stage_tiles[st]
                    j = s % 4
                    u = s // 4
                    r1_s = r_sb[:, j * 64:(j + 1) * 64]
                    r2_s = r_sb[:, (4 + j) * 64:(5 + j) * 64]
                    osl = slice(64 * u, 64 * u + 64)
                    nc.tensor.matmul(m1_stage[osl, :], r1_s, a_p[:, h, :],
                                     start=(j == 0), stop=(j == 3),
                                     skip_group_check=True)
                    nc.tensor.matmul(m2_stage[osl, :], r2_s, a2_p[:, h, :],
                                     start=(j == 0), stop=(j == 3),
                                     skip_group_check=True)
                    if s == STAGE - 1:
                        emit_epilogue(st)

            for p in range(NP + LAG):
                if p - LAG >= 0:
                    emit_back(p - LAG)
                if p < NP:
                    emit_front(p)

    nc.finalize()
    return nc


_GRAPH = None

def _get_graph():
    global _GRAPH
    if _GRAPH is None:
        _GRAPH = build_graph()
    return _GRAPH


def make_in_maps(X: np.ndarray):
    E_np, R_np = _quad_consts()
    Xp = np.zeros((2, NPAD), dtype=np.float32)
    Xp[:, :NTOT] = X
    in_maps = []
    for i in range(NCORES):
        shard = np.ascontiguousarray(Xp[:, i * NC_ELEM:(i + 1) * NC_ELEM])
        in_maps.append({"X": shard, "EXP": E_np, "RED": R_np})
    return in_maps


def kernel(X) -> np.ndarray:
    X = np.asarray(X, dtype=np.float32)
    assert X.shape == (2, NTOT)
    nc = _get_graph()
    res = run_bass_kernel_spmd(nc, make_in_maps(X), core_ids=list(range(NCORES)))
    out = np.concatenate([r["out"] for r in res.results], axis=1)
    return np.ascontiguousarray(out[:, :NTOT])


if __name__ == "__main__":
    rng = np.random.default_rng(0)
    X = rng.random((2, NTOT), dtype=np.float32)
    y = kernel(X)
    print("out shape", y.shape, y.dtype)


# revision 37
# speedup vs baseline: 1.1852x; 1.1852x over previous
import sys
sys.path.insert(0, '/opt/trn_rl_repo')
import numpy as np
import ml_dtypes

import concourse.bass as bass
import concourse.tile as tile
from concourse import bacc, mybir
from concourse.bass_utils import run_bass_kernel_spmd

# ---------------- problem constants (hardcoded per spec) ----------------
NTOT = 1_000_000          # total elements (X is [2, NTOT])
NCORES = 8
Q = 8                     # quadrature nodes (optimized for tanh/ADF, see _quad_consts)
G = 128 // Q              # element groups packed per partition column (16)
F = 512                   # free-dim elements per group per matmul (1 PSUM bank fp32)
EPT = G * F               # elements per tile (8192)
NC_ELEM = 131072          # per-core padded element count
T = NC_ELEM // EPT        # tiles per core (16)
STAGE = 128 // G          # tiles per output stage (8)
NSTAGES = T // STAGE      # 2
CH = NC_ELEM // (128 * F) # phase-1 chunks (2)
NPAD = NC_ELEM * NCORES
LAG = 2                   # software pipeline depth in tile-pairs

F32 = mybir.dt.float32
BF16 = mybir.dt.bfloat16

# 8-node symmetric quadrature for E[tanh(mu + sqrt2 s x)] / E[tanh^2], jointly
# optimized offline over mu in [0,1], s^2 in [0,1] with nodes AND weights
# constrained to the bf16 grid (greedy sequential quantization); separate
# weight sets for the first and second moment. ~3.1e-4 frob error vs the
# 128-node Gauss-Hermite reference (Gauss-Hermite-8 itself gives 7.9e-3).
_XH = [1.96875, 1.25, 0.703125, 0.2275390625]          # descending half-nodes
_W1H = [0.0096435546875, 0.0732421875, 0.1708984375, 0.24609375]
_W2H = [0.01055908203125, 0.0712890625, 0.1728515625, 0.2451171875]


def _quad_consts():
    x = np.array([-v for v in _XH] + _XH[::-1], dtype=np.float64)   # ascending
    w1 = np.array(_W1H + _W1H[::-1], dtype=np.float64)
    w2 = np.array(_W2H + _W2H[::-1], dtype=np.float64)

    # expansion lhsT: [2G, 128]; rhs partition p = g*2 + j (j: 0=mu, 1=std')
    # z partition m = g*Q + q
    E = np.zeros((2 * G, 128), dtype=np.float32)
    for m in range(128):
        g, q = divmod(m, Q)
        E[g * 2 + 0, m] = 1.0
        E[g * 2 + 1, m] = x[q]
    # reduction lhsT (M=64, weighted): slot j = s % 4 selects which 16-col band
    # carries the weights; out partition base is 64*(s//4).
    # RED packs [R1_0..R1_3 | R2_0..R2_3], each [128, 64].
    R = np.zeros((128, 8 * 64), dtype=np.float32)
    for j in range(4):
        for k in range(128):
            g, q = divmod(k, Q)
            R[k, j * 64 + j * G + g] = w1[q]
            R[k, (4 + j) * 64 + j * G + g] = w2[q]
    E4 = np.vstack([E, E, E, E])  # rhs slices at base partitions 0/32/64/96 reuse it
    return E4.astype(ml_dtypes.bfloat16), R.astype(ml_dtypes.bfloat16)


def _dram_ap(t_ap: bass.AP, offset: int, pattern) -> bass.AP:
    return bass.AP(tensor=t_ap.tensor, offset=offset, ap=[list(p) for p in pattern])


def build_graph():
    nc = bacc.Bacc("TRN2", target_bir_lowering=False, debug=False, num_devices=NCORES)
    X = nc.dram_tensor("X", [2, NC_ELEM], F32, kind="ExternalInput").ap()
    EXP = nc.dram_tensor("EXP", [8 * G, 128], BF16, kind="ExternalInput").ap()
    RED = nc.dram_tensor("RED", [128, 8 * 64], BF16, kind="ExternalInput").ap()
    OUT = nc.dram_tensor("out", [2, NC_ELEM], F32, kind="ExternalOutput").ap()

    with tile.TileContext(nc) as tc:
        with tc.tile_pool(name="consts", bufs=1) as consts, \
             tc.tile_pool(name="phase1", bufs=1) as ph1, \
             tc.tile_pool(name="rhs", bufs=3) as rpool, \
             tc.tile_pool(name="acts", bufs=2) as apool, \
             tc.tile_pool(name="stage", bufs=2) as spool, \
             tc.tile_pool(name="zps", bufs=2, space="PSUM") as zpool, \
             tc.tile_pool(name="mps", bufs=2, space="PSUM") as mpool:

            mu_f = ph1.tile([128, CH, F], F32)
            var_f = ph1.tile([128, CH, F], F32)
            for c in range(CH):
                nc.sync.dma_start(mu_f[:, c, :],
                                  _dram_ap(X, c * 128 * F, [[F, 128], [1, F]]))
                nc.gpsimd.dma_start(var_f[:, c, :],
                                    _dram_ap(X, NC_ELEM + c * 128 * F, [[F, 128], [1, F]]))

            e_sb = consts.tile([8 * G, 128], BF16)
            nc.scalar.dma_start(e_sb[:], EXP)
            r_sb = consts.tile([128, 8 * 64], BF16)
            nc.scalar.dma_start(r_sb[:], RED)
            wtiny = consts.tile([128, F], BF16)
            nc.vector.memset(wtiny[:], 0.001)

            # ---- warmup: open the PE clock gate while inputs stream in;
            # the last few depend on phase-1 data so PE activity continues
            # seamlessly into the first real z-matmul (no re-throttle gap).
            wm = zpool.tile([128, 2, F], F32, tag="z")
            for _ in range(12):
                nc.tensor.matmul(wm[:, 0, :], wtiny[:, 0:128], wtiny[:],
                                 start=True, stop=True, skip_group_check=True)

            # ---- phase 1: load X; msd[:, 0]=mu (bf16), msd[:, 1]=sqrt(2*var) (bf16)
            msd = ph1.tile([128, 2, CH, F], BF16)
            for c in range(CH):
                nc.vector.tensor_copy(msd[:, 0, c, :], mu_f[:, c, :])
                nc.scalar.activation(msd[:, 1, c, :], var_f[:, c, :],
                                     mybir.ActivationFunctionType.Sqrt, scale=2.0)

            for _ in range(4):
                nc.tensor.matmul(wm[:, 1, :], wtiny[0:2 * G, 0:128],
                                 msd[0:2 * G, 0, 0, 0:F].bitcast(BF16),
                                 start=True, stop=True, skip_group_check=True)

            # ---- main loop: software-pipelined; tile-pairs share one 2-bank
            # PSUM z tile so ACT/DVE process [128, 2F] spans.
            NP = T // 2
            z_tiles = [None] * NP
            stage_tiles = {}

            def emit_front(p):
                # tiles 2p, 2p+1 sit at contiguous partition ranges of msd
                s0 = (2 * p) % STAGE
                c = (2 * p) // STAGE
                rhs_p = rpool.tile([4 * G, F], BF16, tag="rhs")
                nc.gpsimd.dma_start(rhs_p[:], msd[s0 * G:(s0 + 2) * G, :, c, :])
                z_p = zpool.tile([128, 2, F], F32, tag="z")
                for h in range(2):
                    b = h * 2 * G
                    nc.tensor.matmul(z_p[:, h, :],
                                     e_sb[b:b + 2 * G, :],
                                     rhs_p[b:b + 2 * G, :],
                                     start=True, stop=True, skip_group_check=True,
                                     tile_position=(b, 0))
                z_tiles[p] = z_p

            def emit_epilogue(st):
                m1_stage, m2_stage = stage_tiles[st]
                m1_sb = spool.tile([128, F], F32, tag="m1sb")
                nc.vector.tensor_copy(m1_sb[:], m1_stage[:])
                sq = spool.tile([128, F], F32, tag="sq")
                nc.vector.tensor_mul(sq[:], m1_sb[:], m1_sb[:])
                var_t = spool.tile([128, F], F32, tag="var")
                nc.vector.tensor_sub(var_t[:], m2_stage[:], sq[:])
                off = st * 128 * F
                nc.sync.dma_start(_dram_ap(OUT, off, [[F, 128], [1, F]]), m1_sb[:])
                nc.scalar.dma_start(_dram_ap(OUT, NC_ELEM + off, [[F, 128], [1, F]]), var_t[:])

            def emit_back(p):
                z_p = z_tiles[p]
                a_p = apool.tile([128, 2, F], BF16, tag="a")
                nc.scalar.activation(a_p[:], z_p[:], mybir.ActivationFunctionType.Tanh)
                a2_p = apool.tile([128, 2, F], BF16, tag="a2")
                nc.vector.tensor_mul(a2_p[:], a_p[:], a_p[:])
                for h in range(2):
                    t = 2 * p + h
                    st, s = divmod(t, STAGE)
                    if s == 0:
                        m1s_new = mpool.tile([128, F], F32, tag="m1s")
                        m2s_new = mpool.tile([128, F], F32, tag="m2s")
                        stage_tiles[st] = (m1s_new, m2s_new)
                    m1_stage, m2_stage = stage_tiles[st]
                    j = s % 4
                    u = s // 4
                    r1_s = r_sb[:, j * 64:(j + 1) * 64]
                    r2_s = r_sb[:, (4 + j) * 64:(5 + j) * 64]
                    osl = slice(64 * u, 64 * u + 64)
                    nc.tensor.matmul(m1_stage[osl, :], r1_s, a_p[:, h, :],
                                     start=(j == 0), stop=(j == 3),
                                     skip_group_check=True)
                    nc.tensor.matmul(m2_stage[osl, :], r2_s, a2_p[:, h, :],
                                     start=(j == 0), stop=(j == 3),
                                     skip_group_check=True)
                    if s == STAGE - 1:
                        emit_epilogue(st)

            for p in range(NP + LAG):
                if p < NP:
                    emit_front(p)
                if p - LAG >= 0:
                    emit_back(p - LAG)

    nc.finalize()
    return nc


_GRAPH = None

def _get_graph():
    global _GRAPH
    if _GRAPH is None:
        _GRAPH = build_graph()
    return _GRAPH


def make_in_maps(X: np.ndarray):
    E_np, R_np = _quad_consts()
    Xp = np.zeros((2, NPAD), dtype=np.float32)
    Xp[:, :NTOT] = X
    in_maps = []
    for i in range(NCORES):
        shard = np.ascontiguousarray(Xp[:, i * NC_ELEM:(i + 1) * NC_ELEM])
        in_maps.append({"X": shard, "EXP": E_np, "RED": R_np})
    return in_maps


def kernel(X) -> np.ndarray:
    X = np.asarray(X, dtype=np.float32)
    assert X.shape == (2, NTOT)
    nc = _get_graph()
    res = run_bass_kernel_spmd(nc, make_in_maps(X), core_ids=list(range(NCORES)))
    out = np.concatenate([r["out"] for r in res.results], axis=1)
    return np.ascontiguousarray(out[:, :NTOT])


if __name__ == "__main__":
    rng = np.random.default_rng(0)
    X = rng.random((2, NTOT), dtype=np.float32)
    y = kernel(X)
    print("out shape", y.shape, y.dtype)
